# revision 6
# baseline (speedup 1.0000x reference)
"""Trainium2 Bass kernel for CoocOpModel.

out[b,s,z] = sum_{i,j} func[b,s,i] * cooc[i,j,z] * arg[b,s,j]
  with func = func_and_arg[..., :128], arg = func_and_arg[..., 128:]

Shapes (hardcoded): func_and_arg [4,1024,256] f32, cooccurrences [128,128,128] f32,
out [4,1024,128] f32.  D = 128, tokens T = 4096.

Strategy: data-parallel over tokens across 8 cores (512 tokens/core);
cooccurrence tensor replicated per core (fp16).

Per-core math, with t = local token index (512), i/j/z in [0,128):
  out_T[z, t] = sum_i  C_i^T @ G_i        (accumulated in one PSUM bank)
  C_i[j, z]   = cooc[i, j, z]             (stationary operand, fp16)
  G_i[j, t]   = arg_T[j, t] * func_T[i, t]  (moving operand, fp16)

i's are processed in groups of GRP=8:
  - one broadcast-DMA materializes f_exp_g[j, (k,t)] = func_T[8g+k, t]
    (replicated across the 128 j-partitions)
  - one DVE tensor-tensor multiply builds G for the whole group:
    g_g = a_rep * f_exp_g, where a_rep[j, (k,t)] = arg_T[j, t] (built once
    by a replicating DMA)
  - 8 accumulating matmuls consume it (stationary = per-group cooc tile)
"""

import sys

sys.path.insert(0, "/opt/trn_rl_repo")

import numpy as np
import ml_dtypes
from contextlib import ExitStack

import concourse.bass as bass
import concourse.tile as tile
from concourse import bacc, mybir
from concourse.bass_utils import run_bass_kernel_spmd

BF16 = mybir.dt.float16
F32 = mybir.dt.float32
NP_BF16 = np.float16

N_CORES = 8
D = 128
T_TOTAL = 4096
T_CORE = T_TOTAL // N_CORES  # 512
GRP = 8
N_GRP = D // GRP

_NC_CACHE = None


def _build():
    nc = bacc.Bacc("TRN2", target_bir_lowering=False, debug=False, num_devices=N_CORES)

    f_t = nc.dram_tensor("f_t", [D, T_CORE], BF16, kind="ExternalInput").ap()
    a_t = nc.dram_tensor("a_t", [D, T_CORE], BF16, kind="ExternalInput").ap()
    # c2[j, i*128 + z] = cooc[i, j, z]
    c2 = nc.dram_tensor("c2", [D, D * D], BF16, kind="ExternalInput").ap()
    out_t = nc.dram_tensor("out_t", [D, T_CORE], F32, kind="ExternalOutput").ap()

    with tile.TileContext(nc) as tc:
        with ExitStack() as ctx:
            const_pool = ctx.enter_context(tc.tile_pool(name="const", bufs=1))
            fexp_pool = ctx.enter_context(tc.tile_pool(name="fexp", bufs=3))
            g_pool = ctx.enter_context(tc.tile_pool(name="g", bufs=3))
            out_pool = ctx.enter_context(tc.tile_pool(name="out", bufs=1))
            psum_pool = ctx.enter_context(
                tc.tile_pool(name="psum", bufs=1, space="PSUM")
            )

            # arg_T in SBUF; the TT re-reads it per k via a free-step-0 AP.
            a_sb = const_pool.tile([D, T_CORE], BF16, tag="a")
            nc.sync.dma_start(a_sb[:], a_t[:, :])
            a_ap = a_sb[:]
            a_rep_view = bass.AP(
                a_ap.tensor, a_ap.offset, [a_ap.ap[0], [0, GRP], [1, T_CORE]]
            )

            ps = psum_pool.tile([D, T_CORE], F32)
            c_tiles = []
            for g in range(N_GRP):
                # per-group cooc tile: c_sb[g][j, (k, z)] = cooc[8g+k, j, z]
                c_sb = const_pool.tile([D, GRP * D], BF16, tag=f"c{g}")
                eng = nc.scalar if g % 2 == 0 else nc.sync
                eng.dma_start(c_sb[:], c2[:, g * GRP * D : (g + 1) * GRP * D])
                c_tiles.append(c_sb)

                # f_exp[j, (k, t)] = func_T[8g+k, t], replicated over j
                f_exp = fexp_pool.tile([D, GRP * T_CORE], BF16, tag="fexp")
                f_src = bass.AP(
                    f_t.tensor,
                    g * GRP * T_CORE,
                    [[0, D], [T_CORE, GRP], [1, T_CORE]],
                )
                eng = nc.sync if g % 2 == 0 else nc.scalar
                eng.dma_start(f_exp[:], f_src)

                gt = g_pool.tile([D, GRP * T_CORE], BF16, tag="g")
                nc.vector.tensor_mul(gt[:], a_rep_view, f_exp[:])

                for k in range(GRP):
                    i = g * GRP + k
                    nc.tensor.matmul(
                        ps[:],
                        c_tiles[g][:, k * D : (k + 1) * D],
                        gt[:, k * T_CORE : (k + 1) * T_CORE],
                        start=(i == 0),
                        stop=(i == D - 1),
                    )

            o_sb = out_pool.tile([D, T_CORE], F32, tag="o")
            nc.vector.tensor_copy(o_sb[:], ps[:])
            nc.sync.dma_start(out_t[:, :], o_sb[:])

    nc.compile()
    return nc


def _get_nc():
    global _NC_CACHE
    if _NC_CACHE is None:
        _NC_CACHE = _build()
    return _NC_CACHE


def _prep_in_maps(func_and_arg, cooccurrences):
    fa = np.asarray(func_and_arg, dtype=np.float32).reshape(T_TOTAL, 2 * D)
    c2 = (
        np.ascontiguousarray(
            np.asarray(cooccurrences, dtype=np.float32).transpose(1, 0, 2)
        )
        .reshape(D, D * D)
        .astype(NP_BF16)
    )
    in_maps = []
    for c in range(N_CORES):
        s = fa[c * T_CORE : (c + 1) * T_CORE]  # [512, 256]
        f_tc = np.ascontiguousarray(s[:, :D].T).astype(NP_BF16)  # [128 i, 512 t]
        a_tc = np.ascontiguousarray(s[:, D:].T).astype(NP_BF16)  # [128 j, 512 t]
        in_maps.append({"f_t": f_tc, "a_t": a_tc, "c2": c2})
    return in_maps


def kernel(func_and_arg: np.ndarray, cooccurrences: np.ndarray) -> np.ndarray:
    assert func_and_arg.shape == (4, 1024, 2 * D)
    assert cooccurrences.shape == (D, D, D)

    in_maps = _prep_in_maps(func_and_arg, cooccurrences)
    nc = _get_nc()
    res = run_bass_kernel_spmd(nc, in_maps, core_ids=list(range(N_CORES)))

    # out_t per core: [z=128, t=512] -> [t, z]; concat over cores -> [4096, 128]
    outs = [res.results[c]["out_t"].T for c in range(N_CORES)]
    out = np.concatenate(outs, axis=0).reshape(4, 1024, D).astype(np.float32)
    return out
